# revision 2
# baseline (speedup 1.0000x reference)
"""Multi-head attention (no softmax) on 8 TRN2 NeuronCores.

Problem: x[2,2048,1024], per-head Wq/Wk/Wv[16,64,1024] + biases.
    q = einsum('bsd,hed->bhse', x, Wq) + bq   (same for k, v)
    out = ((q @ k^T) * E^-0.5) @ v, heads concatenated on feature dim.

Key algebraic fact: there is NO softmax, so
    (q k^T * norm) v = q @ (norm * (k^T v))
which collapses the O(S^2) attention into a 64x64 (per head) matmul.

Sharding: tensor-parallel over heads - core c owns heads 2c, 2c+1.
Each core:
  phase 1: project QT/KT/VT = W @ x^T in [feature(128), seq(4096)] layout
           (fp32r matmuls, N=512 moving dim -> full PE speed).
           norm is folded into Wq/bq on the host; biases are per-partition
           adds fused into the PSUM->SBUF copies.
  phase 2: per batch b: PE-transpose K/V tiles to [seq, feat] and
           accumulate M_b = K^T V [128,128] in PSUM over 16 seq-chunks;
           copy the two diagonal 64x64 head blocks into a zeroed SBUF
           tile (cross-head blocks of M are garbage and must be dropped).
  phase 3: outT[:, s-chunk] = M_b(blockdiag).T @ QT[:, s-chunk].
Host gathers: out[b, s, c*128:(c+1)*128] = outT_c[:, b*2048+s].T
"""

import numpy as np

import concourse.bacc as bacc
import concourse.tile as tile
import concourse.mybir as mybir
from concourse import bass2jax

B, S, D, H = 2, 2048, 1024, 16
E = 64          # head dim
NCORES = 8
HL = H // NCORES            # heads per core (2)
EP = HL * E                 # packed feature dim per core (128)
SB = B * S                  # flattened seq (4096)
P = 128                     # partitions
DC = D // P                 # d chunks (8)
SC = 512                    # seq chunk for N=512 matmuls
NSC = SB // SC              # 8 seq chunks
TC = SB // P                # 32 transpose chunks (16 per batch)
NORM = float(E) ** -0.5

F32 = mybir.dt.float32
F32R = mybir.dt.float32r

_compiled = None


def _build():
    nc = bacc.Bacc("TRN2", target_bir_lowering=False, debug=False)

    x_d = nc.dram_tensor("x", [DC, NSC, P, SC], F32R, kind="ExternalInput").ap()
    wq_d = nc.dram_tensor("wq", [P, DC, P], F32R, kind="ExternalInput").ap()
    wk_d = nc.dram_tensor("wk", [P, DC, P], F32R, kind="ExternalInput").ap()
    wv_d = nc.dram_tensor("wv", [P, DC, P], F32R, kind="ExternalInput").ap()
    bq_d = nc.dram_tensor("bq", [P, 1], F32, kind="ExternalInput").ap()
    bk_d = nc.dram_tensor("bk", [P, 1], F32, kind="ExternalInput").ap()
    bv_d = nc.dram_tensor("bv", [P, 1], F32, kind="ExternalInput").ap()
    id_d = nc.dram_tensor("ident", [P, P], F32R, kind="ExternalInput").ap()
    out_d = nc.dram_tensor("outT", [P, SB], F32, kind="ExternalOutput").ap()

    with tile.TileContext(nc) as tc:
        with (
            tc.tile_pool(name="consts", bufs=1) as consts,
            tc.tile_pool(name="xs", bufs=18) as xs_pool,
            tc.tile_pool(name="qkv", bufs=1) as qkv_pool,
            tc.tile_pool(name="kv", bufs=4) as kv_pool,
            tc.tile_pool(name="mt", bufs=1) as mt_pool,
            tc.tile_pool(name="ot", bufs=3) as ot_pool,
            tc.tile_pool(name="pproj", bufs=3, space="PSUM") as pproj,
            tc.tile_pool(name="ptr", bufs=2, space="PSUM") as ptr,
            tc.tile_pool(name="pm", bufs=2, space="PSUM") as pm,
        ):
            # ---- constants ----
            w_tiles = {}
            for name, d in (("wq", wq_d), ("wk", wk_d), ("wv", wv_d)):
                wt = consts.tile([P, DC, P], F32R, tag=name, name=f"{name}_t")
                nc.sync.dma_start(wt[:], d[:])
                w_tiles[name] = wt
            b_tiles = {}
            for name, d in (("bq", bq_d), ("bk", bk_d), ("bv", bv_d)):
                bt = consts.tile([P, 1], F32, tag=name, name=f"{name}_t")
                nc.sync.dma_start(bt[:], d[:])
                b_tiles[name] = bt
            ident = consts.tile([P, P], F32R, tag="ident")
            nc.sync.dma_start(ident[:], id_d[:])

            # ---- phase 1: QT/KT/VT projections ----
            qt = qkv_pool.tile([P, SB], F32R, tag="qt")
            kt = qkv_pool.tile([P, SB], F32R, tag="kt")
            vt = qkv_pool.tile([P, SB], F32R, tag="vt")
            dest = {"wq": (qt, "bq"), "wk": (kt, "bk"), "wv": (vt, "bv")}

            for j in range(NSC):
                xs = []
                for i in range(DC):
                    xt = xs_pool.tile([P, SC], F32R, tag="xs", name=f"x_{i}_{j}")
                    nc.sync.dma_start(xt[:], x_d[i, j])
                    xs.append(xt)
                for wname in ("wq", "wk", "wv"):
                    big, bname = dest[wname]
                    ps = pproj.tile([P, SC], F32, tag="proj", name=f"ps_{wname}_{j}")
                    for i in range(DC):
                        nc.tensor.matmul(
                            ps[:], w_tiles[wname][:, i, :], xs[i][:],
                            start=(i == 0), stop=(i == DC - 1),
                        )
                    sl = big[:, j * SC:(j + 1) * SC]
                    if wname == "wv":
                        # offload one of the three copies to ScalarE
                        nc.scalar.activation(
                            sl, ps[:], mybir.ActivationFunctionType.Identity,
                            bias=b_tiles[bname][:],
                        )
                    else:
                        nc.vector.tensor_scalar_add(sl, ps[:], b_tiles[bname][:])

            # ---- phase 2: M_b = K^T V via PE transposes ----
            m_tiles = []
            for b in range(B):
                mps = pm.tile([P, P], F32, tag="m", name=f"mps_{b}")
                for t in range(TC // B):
                    jb = b * (TC // B) + t
                    sl = slice(jb * P, (jb + 1) * P)
                    ktp = ptr.tile([P, P], F32R, tag="tr", name=f"ktp_{jb}")
                    nc.tensor.transpose(ktp[:], kt[:, sl], ident[:])
                    k_sb = kv_pool.tile([P, P], F32R, tag="k_sb", name=f"k_sb_{jb}")
                    nc.scalar.copy(k_sb[:], ktp[:])
                    vtp = ptr.tile([P, P], F32R, tag="tr", name=f"vtp_{jb}")
                    nc.tensor.transpose(vtp[:], vt[:, sl], ident[:])
                    v_sb = kv_pool.tile([P, P], F32R, tag="v_sb", name=f"v_sb_{jb}")
                    nc.vector.tensor_copy(v_sb[:], vtp[:])
                    nc.tensor.matmul(
                        mps[:], k_sb[:], v_sb[:],
                        start=(t == 0), stop=(t == TC // B - 1),
                    )
                mt = mt_pool.tile([P, P], F32R, tag=f"mt{b}", name=f"mt_{b}")
                # zero-fill without InstMemset (walrus rejects f32r memset)
                nc.vector.tensor_scalar_mul(mt[:], ident[:], 0.0)
                nc.vector.tensor_copy(mt[0:E, 0:E], mps[0:E, 0:E])
                nc.vector.tensor_copy(mt[E:P, E:P], mps[E:P, E:P])
                m_tiles.append(mt)

            # ---- phase 3: outT = M_b.T @ QT ----
            for b in range(B):
                for scj in range(NSC // B):
                    j = b * (NSC // B) + scj
                    sl = slice(j * SC, (j + 1) * SC)
                    ps = pproj.tile([P, SC], F32, tag="proj", name=f"ops_{j}")
                    nc.tensor.matmul(ps[:], m_tiles[b][:], qt[:, sl],
                                     start=True, stop=True)
                    ot = ot_pool.tile([P, SC], F32, tag="ot", name=f"ot_{j}")
                    nc.vector.tensor_copy(ot[:], ps[:])
                    nc.sync.dma_start(out_d[:, sl], ot[:])

    nc.compile()
    return nc


def _prep_inputs(x, Wq, Wk, Wv, bq, bk, bv):
    """Host-side shard + layout prep. Returns per-core input maps."""
    xf = np.ascontiguousarray(x.reshape(SB, D).T)          # [D, SB]
    x_tiles = np.ascontiguousarray(
        xf.reshape(DC, P, NSC, SC).transpose(0, 2, 1, 3)
    )                                                       # [DC, NSC, P, SC]

    in_maps = []
    for c in range(NCORES):
        hs = slice(HL * c, HL * (c + 1))
        wq_c = (Wq[hs].reshape(EP, D) * NORM).astype(np.float32)
        wk_c = Wk[hs].reshape(EP, D).astype(np.float32)
        wv_c = Wv[hs].reshape(EP, D).astype(np.float32)

        def wlayout(w):                                     # [EP, D] -> [P, DC, P]
            return np.ascontiguousarray(w.T.reshape(DC, P, EP).transpose(1, 0, 2))

        in_maps.append({
            "x": x_tiles,
            "wq": wlayout(wq_c),
            "wk": wlayout(wk_c),
            "wv": wlayout(wv_c),
            "bq": (bq[hs].reshape(EP, 1) * NORM).astype(np.float32),
            "bk": bk[hs].reshape(EP, 1).astype(np.float32),
            "bv": bv[hs].reshape(EP, 1).astype(np.float32),
            "ident": np.eye(P, dtype=np.float32),
        })
    return in_maps


def _gather(results):
    out = np.empty((B, S, D), dtype=np.float32)
    for c in range(NCORES):
        oc = results[c]["outT"]                             # [P, SB]
        for b in range(B):
            out[b, :, EP * c:EP * (c + 1)] = oc[:, b * S:(b + 1) * S].T
    return out


def get_compiled():
    global _compiled
    if _compiled is None:
        _compiled = _build()
    return _compiled


def run(in_maps):
    nc = get_compiled()
    return bass2jax.run_bass_via_pjrt(nc, in_maps, n_cores=NCORES)


def kernel(x, Wq, Wk, Wv, bq, bk, bv):
    x = np.asarray(x, dtype=np.float32)
    in_maps = _prep_inputs(
        np.asarray(x, np.float32), np.asarray(Wq, np.float32),
        np.asarray(Wk, np.float32), np.asarray(Wv, np.float32),
        np.asarray(bq, np.float32), np.asarray(bk, np.float32),
        np.asarray(bv, np.float32),
    )
    return _gather(run(in_maps))
